# revision 2
# baseline (speedup 1.0000x reference)
"""NT-Xent (SimCLR) contrastive loss on 8 Trainium2 NeuronCores.

Strategy: rows of the 8192x8192 similarity matrix are sharded across the 8
cores (1024 rows each). Every core receives the full z^T = [256, 8192] raw
embeddings in bf16, column-rotated so that its 1024 rows sit at local
columns [0, 1024). This makes the program identical on every core: the
diagonal (self-similarity) mask lives at local sim[i, i] and the positive
pair at local column i + 4096. Each core normalizes z on-device, computes
its 1024x8192 block of exp(2 * z_i . z_j) fused in PSUM/SBUF (never hitting
HBM), row-reduces, and emits one partial scalar
sum_r(log(denom_r) - pos_r). The host sums the 8 partials and divides by 2N.

All matmul operands are bf16 (1 cyc/row at the PE's full 2.4 GHz clock vs
the ~1.1 GHz effective rate fp32r measured on HW), and the elementwise
normalization runs in bf16 on DVE (2x mode for 2-byte dtypes).
"""

import sys

for _p in ("/opt/trn_rl_repo",):
    if _p not in sys.path:
        sys.path.insert(0, _p)

import ml_dtypes
import numpy as np

import concourse.bass as bass
import concourse.tile as tile
from concourse import bacc, mybir
from concourse.bass_utils import run_bass_kernel_spmd

F32 = mybir.dt.float32
BF16 = mybir.dt.bfloat16
AF = mybir.ActivationFunctionType

N_CORES = 8
N = 4096
D = 256
TWO_N = 2 * N          # 8192 rows/cols of sim
ROWS = TWO_N // N_CORES  # 1024 rows per core
NEG_MASK = -1.0e5      # additive pre-exp diagonal mask; exp underflows to 0

_CACHE = {}
LAST_RESULTS = None


def _build_nc() -> bass.Bass:
    nc = bacc.Bacc("TRN2", num_devices=N_CORES)

    zt_d = nc.dram_tensor("zt", [D, TWO_N], BF16, kind="ExternalInput")
    mask_d = nc.dram_tensor("dmask", [128, 128], F32, kind="ExternalInput")
    out_d = nc.dram_tensor("out", [1, 1], F32, kind="ExternalOutput")
    u_d = nc.dram_tensor("uscratch", [1, TWO_N], BF16)  # internal DRAM bounce

    with tile.TileContext(nc) as tc:
        with (
            tc.tile_pool(name="big", bufs=1) as big,
            tc.tile_pool(name="wsq", bufs=4) as wsq,
            tc.tile_pool(name="wub", bufs=3) as wub,
            tc.tile_pool(name="wsm", bufs=2) as wsm,
            tc.tile_pool(name="wsr", bufs=1) as wsr,
            tc.tile_pool(name="small", bufs=1) as small,
            tc.tile_pool(name="stat", bufs=1) as stat,
            tc.tile_pool(name="ps", bufs=2, space="PSUM") as ps,
        ):
            # Normalized z^T in bf16 for full-rate PE matmuls.
            zt0 = big.tile([128, TWO_N], BF16, tag="zt0")  # dims 0:128
            zt1 = big.tile([128, TWO_N], BF16, tag="zt1")  # dims 128:256
            # Raw z^T staged with whole-row DMAs (128 descriptors each).
            ztr0 = big.tile([128, TWO_N], BF16, tag="ztr0")
            ztr1 = big.tile([128, TWO_N], BF16, tag="ztr1")
            mask_f = small.tile([128, 128], F32, tag="maskf")
            nc.sync.dma_start(out=mask_f[:, :], in_=mask_d.ap()[:, :])
            ones_f = small.tile([128, 1], F32, tag="onesf")
            nc.vector.memset(ones_f[:, :], 1.0)
            ones = small.tile([128, 1], BF16, tag="ones")
            nc.vector.tensor_copy(ones[:, :], ones_f[:, :])

            # per-(row-tile, block) partial row sums of exp
            BLOCKS = [1024, 1024, 2048, 2048, 2048]
            OFFS = [sum(BLOCKS[:i]) for i in range(len(BLOCKS))]
            NB = len(BLOCKS)
            rowsums = stat.tile([128, 8 * NB], F32, tag="rsum")

            def load(b):
                o, w = OFFS[b], BLOCKS[b]
                sl = slice(o, o + w)
                nc.sync.dma_start(out=ztr0[:, sl], in_=zt_d.ap()[0:128, sl])
                nc.sync.dma_start(out=ztr1[:, sl], in_=zt_d.ap()[128:256, sl])

            def prologue(b):
                o, w = OFFS[b], BLOCKS[b]
                sl = slice(o, o + w)
                sq0 = wsq.tile([128, w], BF16, tag="sq")
                sq1 = wsq.tile([128, w], BF16, tag="sq")
                nc.vector.tensor_mul(sq0[:, :], ztr0[:, sl], ztr0[:, sl])
                nc.vector.tensor_mul(sq1[:, :], ztr1[:, sl], ztr1[:, sl])

                nrm = ps.tile([1, w], F32, tag="mm")
                for bb in range(w // 512):
                    bs = slice(bb * 512, (bb + 1) * 512)
                    for ki, sqk in enumerate((sq0, sq1)):
                        nc.tensor.matmul(
                            nrm[0:1, bs], ones[:, :], sqk[:, bs],
                            start=(ki == 0), stop=(ki == 1),
                        )
                # evacuate [1, w] psum -> sbuf, then DMA-transpose into w/64
                # partitions of sst: sst[p, i] = ssq(col o+p*64+i)
                ssqr = wsr.tile([1, w], F32, tag="ssqr")
                nc.vector.tensor_copy(ssqr[0:1, :], nrm[0:1, :])
                np_ = w // 64
                sstb = wsm.tile([np_, 64], F32, tag="sstb")
                nc.sync.dma_start(out=sstb[0:np_, :], in_=ssqr[0:1, :])
                # u = exp(-0.5 * ln(ssq)) = 1/sqrt(ssq); stays in the exp+ln
                # activation table set (no table switching).
                lnt = wsm.tile([np_, 64], F32, tag="lnt")
                ut = wsm.tile([np_, 64], BF16, tag="ut")
                nc.scalar.activation(lnt[:, :], sstb[0:np_, :], AF.Ln)
                nc.scalar.activation(ut[:, :], lnt[:, :], AF.Exp, scale=-0.5)
                # scatter u back to DRAM in column order, then broadcast-read
                # across all 128 partitions.
                u_out = bass.AP(
                    tensor=u_d.ap().tensor,
                    offset=o,
                    ap=[[64, np_], [1, 64]],
                )
                nc.sync.dma_start(out=u_out, in_=ut[:, :])
                ubc = wub.tile([128, w], BF16, tag="ubc")
                u_sl = u_d.ap()[0:1, sl]
                u_bcast = bass.AP(
                    tensor=u_sl.tensor,
                    offset=u_sl.offset,
                    ap=[[0, 128]] + list(u_sl.ap[1:]),
                )
                nc.sync.dma_start(out=ubc[:, :], in_=u_bcast)
                return ubc

            def mults(b, ubc):
                o, w = OFFS[b], BLOCKS[b]
                sl = slice(o, o + w)
                nc.vector.tensor_mul(zt0[:, sl], ztr0[:, sl], ubc[:, :])
                nc.vector.tensor_mul(zt1[:, sl], ztr1[:, sl], ubc[:, :])

            def qpass(b):
                # all 8 row-tiles against column block b
                o, w = OFFS[b], BLOCKS[b]
                for rt in range(ROWS // 128):
                    r0 = rt * 128
                    pq = ps.tile([128, w], F32, tag="mm")
                    has_mask = o <= r0 < o + w
                    for ki, zk in enumerate((zt0, zt1)):
                        lhsT = zk[:, r0 : r0 + 128]
                        for bb in range(w // 512):
                            off = o + bb * 512
                            nc.tensor.matmul(
                                pq[:, bb * 512 : (bb + 1) * 512],
                                lhsT,
                                zk[:, off : off + 512],
                                start=(ki == 0),
                                stop=(ki == 1),
                            )
                    if has_mask:
                        # diagonal crossing at local cols r0..r0+128
                        mo = r0 - o
                        nc.vector.tensor_add(
                            pq[:, mo : mo + 128], pq[:, mo : mo + 128],
                            mask_f[:, :],
                        )
                    # exp(2 * sim) in place in PSUM; row partial sums out.
                    nc.scalar.activation(
                        pq[:, :],
                        pq[:, :],
                        AF.Exp,
                        scale=2.0,
                        accum_out=rowsums[:, rt * NB + b : rt * NB + b + 1],
                    )

            def pos_pass():
                # pos_dot[i] = z_i . z_{i+4096}, local rows 0..1023
                posps = ps.tile([1, ROWS], F32, tag="mm")
                for ki, zk in enumerate((zt0, zt1)):
                    prod = wsq.tile([128, ROWS], BF16, tag="sq")
                    nc.vector.tensor_mul(
                        prod[:, :], zk[:, 0:ROWS], zk[:, N : N + ROWS]
                    )
                    for bb in range(ROWS // 512):
                        bs = slice(bb * 512, (bb + 1) * 512)
                        nc.tensor.matmul(
                            posps[0:1, bs],
                            ones[:, :],
                            prod[:, bs],
                            start=(ki == 0),
                            stop=(ki == 1),
                        )
                pos_tot = stat.tile([1, 1], F32, tag="ptot")
                nc.vector.tensor_reduce(
                    pos_tot[:, :], posps[0:1, :], axis=mybir.AxisListType.X,
                    op=mybir.AluOpType.add,
                )
                return pos_tot

            # staggered emission: prologue(b+1) ahead of mults(b) ahead of
            # qpass(b); loads trickle in just before they are needed.
            # NOTE: every qpass(b) reads stationary columns [0, 1024) =
            # block 0, so mults(0) must precede all qpasses in emission
            # order (Tile orders by emission; a later write is NOT a dep).
            load(0)
            load(1)
            ub = {}
            ub[0] = prologue(0)
            mults(0, ub[0])
            qpass(0)
            load(2)
            ub[1] = prologue(1)
            mults(1, ub[1])
            qpass(1)
            load(3)
            ub[2] = prologue(2)
            mults(2, ub[2])
            load(4)
            qpass(2)
            ub[3] = prologue(3)
            mults(3, ub[3])
            pos_tot = pos_pass()
            qpass(3)
            ub[4] = prologue(4)
            mults(4, ub[4])
            qpass(4)

            # denom per row: sum the NB block partials, then ln and reduce.
            denom = stat.tile([128, 8], F32, tag="den")
            nc.vector.tensor_reduce(
                denom[:, :],
                rowsums[:, :].rearrange("p (a b) -> p a b", b=NB),
                axis=mybir.AxisListType.X,
                op=mybir.AluOpType.add,
            )
            lnden = stat.tile([128, 8], F32, tag="lnd")
            lnsum = stat.tile([128, 1], F32, tag="lns")
            nc.scalar.activation(
                lnden[:, :], denom[:, :], AF.Ln, accum_out=lnsum[:, :]
            )
            finps = ps.tile([1, 1], F32, tag="mm")
            nc.tensor.matmul(
                finps[0:1, 0:1], ones_f[:, :], lnsum[:, :],
                start=True, stop=True,
            )

            # res = sum(ln denom) - 2 * sum(pos_dot)
            res = stat.tile([1, 1], F32, tag="res")
            nc.vector.tensor_scalar_mul(res[:, :], pos_tot[:, :], -2.0)
            nc.vector.tensor_add(res[:, :], res[:, :], finps[0:1, 0:1])
            nc.sync.dma_start(out=out_d.ap()[:, :], in_=res[:, :])

    # The stock insert_act_table_loads pass assigns Exp->exp_and_others and
    # Ln->natural_log, reloading the ACT spline tables (~2.7us each) at every
    # Ln/Exp switch. Both functions live together in
    # natural_log_exp_and_others; load that one set once instead.
    _combined_set_id = _act_set_id_with_exp_and_ln(nc)

    def _single_act_table_load():
        for blk in nc.main_func.blocks:
            insts = list(blk.instructions)
            for i, ins in enumerate(insts):
                if isinstance(ins, mybir.InstActivation):
                    load = mybir.InstLoadActFuncSet(
                        name=nc.get_next_instruction_name(),
                        act_func_set_id=_combined_set_id,
                        ins=[],
                        outs=[],
                    )
                    load.engine = mybir.EngineType.Activation
                    insts.insert(i, load)
                    blk.instructions = insts
                    break

    nc.insert_act_table_loads = _single_act_table_load
    nc.compile()
    return nc


def _act_set_id_with_exp_and_ln(nc) -> int:
    from concourse.hw_specs import get_activation_tables

    tabs = get_activation_tables(nc.m.arch)
    for i, (name, fns) in enumerate(tabs.items()):
        if AF.Exp in fns and AF.Ln in fns:
            return i
    raise RuntimeError("no activation table set with both Exp and Ln")


def _get_nc() -> bass.Bass:
    if "nc" not in _CACHE:
        _CACHE["nc"] = _build_nc()
    return _CACHE["nc"]


def _diag_mask() -> np.ndarray:
    m = np.zeros((128, 128), dtype=np.float32)
    np.fill_diagonal(m, NEG_MASK)
    return m


def kernel(emb_i: np.ndarray, emb_j: np.ndarray) -> np.ndarray:
    global LAST_RESULTS
    z = np.concatenate(
        [np.asarray(emb_i, dtype=np.float32), np.asarray(emb_j, dtype=np.float32)],
        axis=0,
    )  # [8192, 256]
    zt = np.ascontiguousarray(z.T).astype(ml_dtypes.bfloat16)  # [256, 8192]
    dmask = _diag_mask()

    in_maps = []
    for c in range(N_CORES):
        ztc = zt if c == 0 else np.ascontiguousarray(
            np.roll(zt, -c * ROWS, axis=1)
        )
        in_maps.append({"zt": ztc, "dmask": dmask})

    nc = _get_nc()
    LAST_RESULTS = run_bass_kernel_spmd(nc, in_maps, list(range(N_CORES)))
    total = sum(float(r["out"][0, 0]) for r in LAST_RESULTS.results)
    return np.array(total / TWO_N, dtype=np.float32)


# revision 6
# speedup vs baseline: 1.6088x; 1.6088x over previous
"""NT-Xent (SimCLR) contrastive loss on 8 Trainium2 NeuronCores.

Strategy: rows of the 8192x8192 similarity matrix are sharded across the 8
cores (1024 rows each). Every core receives the full z^T = [256, 8192] raw
embeddings in bf16, column-rotated so that its 1024 rows sit at local
columns [0, 1024). This makes the program identical on every core: the
diagonal (self-similarity) mask lives at local sim[i, i] and the positive
pair at local column i + 4096. Each core normalizes z on-device, computes
its 1024x8192 block of exp(2 * z_i . z_j) fused in PSUM/SBUF (never hitting
HBM), row-reduces, and emits one partial scalar
sum_r(log(denom_r) - pos_r). The host sums the 8 partials and divides by 2N.

All matmul operands are bf16 (1 cyc/row at the PE's full 2.4 GHz clock vs
the ~1.1 GHz effective rate fp32r measured on HW), and the elementwise
normalization runs in bf16 on DVE (2x mode for 2-byte dtypes).
"""

import sys

for _p in ("/opt/trn_rl_repo",):
    if _p not in sys.path:
        sys.path.insert(0, _p)

import ml_dtypes
import numpy as np

import concourse.bass as bass
import concourse.tile as tile
from concourse import bacc, mybir
from concourse.bass_utils import run_bass_kernel_spmd

F32 = mybir.dt.float32
BF16 = mybir.dt.bfloat16
AF = mybir.ActivationFunctionType

N_CORES = 8
N = 4096
D = 256
TWO_N = 2 * N          # 8192 rows/cols of sim
ROWS = TWO_N // N_CORES  # 1024 rows per core
NEG_MASK = -1.0e5      # additive pre-exp diagonal mask; exp underflows to 0

_CACHE = {}
LAST_RESULTS = None


def _build_nc() -> bass.Bass:
    nc = bacc.Bacc("TRN2", num_devices=N_CORES)

    zt_d = nc.dram_tensor("zt", [D, TWO_N], BF16, kind="ExternalInput")
    mask_d = nc.dram_tensor("dmask", [128, 128], F32, kind="ExternalInput")
    out_d = nc.dram_tensor("out", [1, 1], F32, kind="ExternalOutput")
    u_d = nc.dram_tensor("uscratch", [1, TWO_N], BF16)  # internal DRAM bounce

    with tile.TileContext(nc) as tc:
        with (
            tc.tile_pool(name="big", bufs=1) as big,
            tc.tile_pool(name="wsq", bufs=4) as wsq,
            tc.tile_pool(name="wub", bufs=3) as wub,
            tc.tile_pool(name="wsm", bufs=6) as wsm,
            tc.tile_pool(name="wsr", bufs=2) as wsr,
            tc.tile_pool(name="small", bufs=1) as small,
            tc.tile_pool(name="stat", bufs=1) as stat,
            tc.tile_pool(name="ps", bufs=2, space="PSUM") as ps,
        ):
            # Normalized z^T in bf16 for full-rate PE matmuls.
            zt0 = big.tile([128, TWO_N], BF16, tag="zt0")  # dims 0:128
            zt1 = big.tile([128, TWO_N], BF16, tag="zt1")  # dims 128:256
            # Raw z^T staged with whole-row DMAs (128 descriptors each).
            ztr0 = big.tile([128, TWO_N], BF16, tag="ztr0")
            ztr1 = big.tile([128, TWO_N], BF16, tag="ztr1")
            mask_f = small.tile([128, 128], F32, tag="maskf")
            nc.sync.dma_start(out=mask_f[:, :], in_=mask_d.ap()[:, :])
            ones_f = small.tile([128, 1], F32, tag="onesf")
            nc.vector.memset(ones_f[:, :], 1.0)
            ones = small.tile([128, 1], BF16, tag="ones")
            nc.vector.tensor_copy(ones[:, :], ones_f[:, :])

            # per-(row-tile, block) partial row sums of exp
            BLOCKS = [1024, 1024, 2048, 2048, 2048]
            OFFS = [sum(BLOCKS[:i]) for i in range(len(BLOCKS))]
            NB = len(BLOCKS)
            rowsums = stat.tile([128, 8 * NB], F32, tag="rsum")

            def load(b):
                o, w = OFFS[b], BLOCKS[b]
                sl = slice(o, o + w)
                nc.sync.dma_start(out=ztr0[:, sl], in_=zt_d.ap()[0:128, sl])
                nc.sync.dma_start(out=ztr1[:, sl], in_=zt_d.ap()[128:256, sl])

            def prologue(b):
                o, w = OFFS[b], BLOCKS[b]
                sl = slice(o, o + w)
                sq0 = wsq.tile([128, w], BF16, tag="sq")
                sq1 = wsq.tile([128, w], BF16, tag="sq")
                nc.vector.tensor_mul(sq0[:, :], ztr0[:, sl], ztr0[:, sl])
                nc.vector.tensor_mul(sq1[:, :], ztr1[:, sl], ztr1[:, sl])

                nrm = ps.tile([1, w], F32, tag="mm")
                for bb in range(w // 512):
                    bs = slice(bb * 512, (bb + 1) * 512)
                    for ki, sqk in enumerate((sq0, sq1)):
                        nc.tensor.matmul(
                            nrm[0:1, bs], ones[:, :], sqk[:, bs],
                            start=(ki == 0), stop=(ki == 1),
                        )
                # evacuate [1, w] psum -> sbuf (DMA cannot read PSUM), then
                # DMA-transpose into w/64 partitions of sst:
                # sst[p, i] = ssq(col o+p*64+i)
                ssqr = wsr.tile([1, w], F32, tag="ssqr")
                nc.vector.tensor_copy(ssqr[0:1, :], nrm[0:1, :])
                np_ = w // 64
                sstb = wsm.tile([np_, 64], F32, tag="sstb")
                nc.sync.dma_start(out=sstb[0:np_, :], in_=ssqr[0:1, :])
                # u = exp(-0.5 * ln(ssq)) = 1/sqrt(ssq); stays in the exp+ln
                # activation table set (no table switching).
                lnt = wsm.tile([np_, 64], F32, tag="lnt")
                ut = wsm.tile([np_, 64], BF16, tag="ut")
                nc.scalar.activation(lnt[:, :], sstb[0:np_, :], AF.Ln)
                nc.scalar.activation(ut[:, :], lnt[:, :], AF.Exp, scale=-0.5)
                # scatter u back to DRAM in column order, then broadcast-read
                # across all 128 partitions.
                u_out = bass.AP(
                    tensor=u_d.ap().tensor,
                    offset=o,
                    ap=[[64, np_], [1, 64]],
                )
                nc.sync.dma_start(out=u_out, in_=ut[:, :])
                ubc = wub.tile([128, w], BF16, tag="ubc")
                u_sl = u_d.ap()[0:1, sl]
                u_bcast = bass.AP(
                    tensor=u_sl.tensor,
                    offset=u_sl.offset,
                    ap=[[0, 128]] + list(u_sl.ap[1:]),
                )
                nc.sync.dma_start(out=ubc[:, :], in_=u_bcast)
                return ubc

            def mults(b, ubc):
                o, w = OFFS[b], BLOCKS[b]
                sl = slice(o, o + w)
                nc.vector.tensor_mul(zt0[:, sl], ztr0[:, sl], ubc[:, :])
                nc.vector.tensor_mul(zt1[:, sl], ztr1[:, sl], ubc[:, :])

            def qpass(b):
                # all 8 row-tiles against column block b
                o, w = OFFS[b], BLOCKS[b]
                for rt in range(ROWS // 128):
                    r0 = rt * 128
                    pq = ps.tile([128, w], F32, tag="mm")
                    has_mask = o <= r0 < o + w
                    for ki, zk in enumerate((zt0, zt1)):
                        lhsT = zk[:, r0 : r0 + 128]
                        for bb in range(w // 512):
                            off = o + bb * 512
                            nc.tensor.matmul(
                                pq[:, bb * 512 : (bb + 1) * 512],
                                lhsT,
                                zk[:, off : off + 512],
                                start=(ki == 0),
                                stop=(ki == 1),
                            )
                    if has_mask:
                        # diagonal crossing at local cols r0..r0+128
                        mo = r0 - o
                        nc.vector.tensor_add(
                            pq[:, mo : mo + 128], pq[:, mo : mo + 128],
                            mask_f[:, :],
                        )
                    # exp(2 * sim) in place in PSUM; row partial sums out.
                    nc.scalar.activation(
                        pq[:, :],
                        pq[:, :],
                        AF.Exp,
                        scale=2.0,
                        accum_out=rowsums[:, rt * NB + b : rt * NB + b + 1],
                    )

            def pos_pass():
                # pos_dot[i] = z_i . z_{i+4096}, local rows 0..1023
                posps = ps.tile([1, ROWS], F32, tag="mm")
                for ki, zk in enumerate((zt0, zt1)):
                    prod = wsq.tile([128, ROWS], BF16, tag="sq")
                    nc.vector.tensor_mul(
                        prod[:, :], zk[:, 0:ROWS], zk[:, N : N + ROWS]
                    )
                    for bb in range(ROWS // 512):
                        bs = slice(bb * 512, (bb + 1) * 512)
                        nc.tensor.matmul(
                            posps[0:1, bs],
                            ones[:, :],
                            prod[:, bs],
                            start=(ki == 0),
                            stop=(ki == 1),
                        )
                pos_tot = stat.tile([1, 1], F32, tag="ptot")
                nc.vector.tensor_reduce(
                    pos_tot[:, :], posps[0:1, :], axis=mybir.AxisListType.X,
                    op=mybir.AluOpType.add,
                )
                return pos_tot

            # staggered emission with TWO blocks of prologue lookahead:
            # each engine executes its instructions in emission order, so
            # block b+1's normalization chain (DVE squares/copies, ACT
            # ln/exp, DMA bounces, DVE mults) must be emitted BEFORE
            # qpass(b)'s 8 big exps or the chain serializes behind them
            # and the PE stalls ~10us per block (observed), which also
            # drops the PE out of its ramped 2.4 GHz p-state.
            load(0)
            load(1)
            load(2)
            ub = {}
            ub[0] = prologue(0)
            mults(0, ub[0])
            ub[1] = prologue(1)
            mults(1, ub[1])
            qpass(0)
            load(3)
            ub[2] = prologue(2)
            mults(2, ub[2])
            qpass(1)
            load(4)
            ub[3] = prologue(3)
            mults(3, ub[3])
            qpass(2)
            ub[4] = prologue(4)
            mults(4, ub[4])
            pos_tot = pos_pass()
            qpass(3)
            qpass(4)

            # denom per row: sum the NB block partials, then ln and reduce.
            denom = stat.tile([128, 8], F32, tag="den")
            nc.vector.tensor_reduce(
                denom[:, :],
                rowsums[:, :].rearrange("p (a b) -> p a b", b=NB),
                axis=mybir.AxisListType.X,
                op=mybir.AluOpType.add,
            )
            lnden = stat.tile([128, 8], F32, tag="lnd")
            lnsum = stat.tile([128, 1], F32, tag="lns")
            nc.scalar.activation(
                lnden[:, :], denom[:, :], AF.Ln, accum_out=lnsum[:, :]
            )
            finps = ps.tile([1, 1], F32, tag="mm")
            nc.tensor.matmul(
                finps[0:1, 0:1], ones_f[:, :], lnsum[:, :],
                start=True, stop=True,
            )

            # res = sum(ln denom) - 2 * sum(pos_dot)
            res = stat.tile([1, 1], F32, tag="res")
            nc.vector.tensor_scalar_mul(res[:, :], pos_tot[:, :], -2.0)
            nc.vector.tensor_add(res[:, :], res[:, :], finps[0:1, 0:1])
            nc.sync.dma_start(out=out_d.ap()[:, :], in_=res[:, :])

    # The stock insert_act_table_loads pass assigns Exp->exp_and_others and
    # Ln->natural_log, reloading the ACT spline tables (~2.7us each) at every
    # Ln/Exp switch. Both functions live together in
    # natural_log_exp_and_others; load that one set once instead.
    _combined_set_id = _act_set_id_with_exp_and_ln(nc)

    def _single_act_table_load():
        for blk in nc.main_func.blocks:
            insts = list(blk.instructions)
            for i, ins in enumerate(insts):
                if isinstance(ins, mybir.InstActivation):
                    load = mybir.InstLoadActFuncSet(
                        name=nc.get_next_instruction_name(),
                        act_func_set_id=_combined_set_id,
                        ins=[],
                        outs=[],
                    )
                    load.engine = mybir.EngineType.Activation
                    insts.insert(i, load)
                    blk.instructions = insts
                    break

    nc.insert_act_table_loads = _single_act_table_load
    nc.compile()
    return nc


def _act_set_id_with_exp_and_ln(nc) -> int:
    from concourse.hw_specs import get_activation_tables

    tabs = get_activation_tables(nc.m.arch)
    for i, (name, fns) in enumerate(tabs.items()):
        if AF.Exp in fns and AF.Ln in fns:
            return i
    raise RuntimeError("no activation table set with both Exp and Ln")


def _get_nc() -> bass.Bass:
    if "nc" not in _CACHE:
        _CACHE["nc"] = _build_nc()
    return _CACHE["nc"]


def _diag_mask() -> np.ndarray:
    m = np.zeros((128, 128), dtype=np.float32)
    np.fill_diagonal(m, NEG_MASK)
    return m


def kernel(emb_i: np.ndarray, emb_j: np.ndarray) -> np.ndarray:
    global LAST_RESULTS
    z = np.concatenate(
        [np.asarray(emb_i, dtype=np.float32), np.asarray(emb_j, dtype=np.float32)],
        axis=0,
    )  # [8192, 256]
    zt = np.ascontiguousarray(z.T).astype(ml_dtypes.bfloat16)  # [256, 8192]
    dmask = _diag_mask()

    in_maps = []
    for c in range(N_CORES):
        ztc = zt if c == 0 else np.ascontiguousarray(
            np.roll(zt, -c * ROWS, axis=1)
        )
        in_maps.append({"zt": ztc, "dmask": dmask})

    nc = _get_nc()
    LAST_RESULTS = run_bass_kernel_spmd(nc, in_maps, list(range(N_CORES)))
    total = sum(float(r["out"][0, 0]) for r in LAST_RESULTS.results)
    return np.array(total / TWO_N, dtype=np.float32)
